# revision 2
# baseline (speedup 1.0000x reference)
"""Trainium2 Bass kernel for a 2-layer tanh RNN + final FC.

Reference computation (PyTorch-style RNN, batch_first):
  layer l: h_t = tanh(x_t @ W_ih^T + b_ih + b_hh + h_{t-1} @ W_hh^T)
  out = h1_T @ W_fc^T + b_fc          (final hidden of layer 1)

Shapes: x [64, 1024, 256], H=512, O=256.

Sharding: data-parallel over batch across 8 cores (8 samples/core);
weights replicated.

Per-core design (fp32 matmuls -> single self-loading PE instructions,
fp16 only where the DMA-xbar transpose requires a 2-byte dtype):

* Input projections (x @ W_ih0^T, out0 @ W_ih1^T) are hoisted out of
  the sequential loop and computed as batched matmuls (N=512) over
  128-step chunks, with layer 1 pipelined one chunk behind layer 0.

* The sequential recurrence keeps the hidden state of BOTH layers in
  one fp32 tile hc[:, t] of shape [128, 8, 8]: hc[p, t, j, b] holds
  layer-0 h[b, 128j+p] (chunk c) for j in 0..3 and layer-1 h[b,
  128(j-4)+p] (chunk c-1) for j in 4..7. One fused step computes both
  layers with a minimal instruction count (the backend's cost is
  dominated by fixed per-instruction overheads, not data size):
    - 4+4 fp32 matmuls  z = h @ W_hh^T  (stationary = h j-slice
      [128,8], moving = W_hh^T k-row [128,512]) into ONE two-bank
      PSUM tile [8,1024] (layer 0 cols 0-511, layer 1 cols 512-1023)
    - 1 ScalarE copy into a [16,1024] fp16 staging tile
    - 1 DMA xbar transpose [16,1024] -> [128,8,16]
    - 1 DVE add (+xW chunk) and 1 ScalarE tanh -> hc[:, t]
"""

import sys

if "/opt/trn_rl_repo" not in sys.path:
    sys.path.insert(0, "/opt/trn_rl_repo")

import numpy as np

import concourse.bacc as bacc
import concourse.mybir as mybir
import concourse.tile as tile
from concourse import bass_utils

F16 = mybir.dt.float16
F32 = mybir.dt.float32
AF = mybir.ActivationFunctionType

N_CORES = 8
B, T, D, H, O = 64, 1024, 256, 512, 256
BC = B // N_CORES  # batch per core

CH = 128  # timesteps per chunk
KH = H // 128  # 4
KD = D // 128  # 2
MO = O // 128  # 2
J = 2 * KH  # 8: combined k-tile axis (j 0-3 layer 0, j 4-7 layer 1)
XP = 16  # xbar transpose partition granule


def build(T=T, CH=CH, reps=1):
    """Build the per-core Bass program. reps>1 re-runs the whole body
    (timing amplification only)."""
    NCH = T // CH
    NB = CH * BC  # columns per chunk in (t, b) order
    SPL = min(512, NB)  # matmul free-dim per split (<= one PSUM bank)
    NS = NB // SPL
    TPS = SPL // BC  # timesteps covered per split

    nc = bacc.Bacc("TRN2", target_bir_lowering=False, debug=False,
                   num_devices=N_CORES)

    xT_d = nc.dram_tensor("xT", [D, T * BC], F16, kind="ExternalInput")
    wih0_d = nc.dram_tensor("wih0T", [D, H], F16, kind="ExternalInput")
    whh0_d = nc.dram_tensor("whh0T", [H, H], F16, kind="ExternalInput")
    wih1_d = nc.dram_tensor("wih1T", [H, H], F16, kind="ExternalInput")
    whh1_d = nc.dram_tensor("whh1T", [H, H], F16, kind="ExternalInput")
    wfc_d = nc.dram_tensor("wfcT", [H, O], F16, kind="ExternalInput")
    b0_d = nc.dram_tensor("b0", [128, KH], F32, kind="ExternalInput")
    b1_d = nc.dram_tensor("b1", [128, KH], F32, kind="ExternalInput")
    bfc_d = nc.dram_tensor("bfc", [128, MO], F32, kind="ExternalInput")
    out_d = nc.dram_tensor("out", [O, BC], F32, kind="ExternalOutput")

    with tile.TileContext(nc) as tc:
        with (
            tc.tile_pool(name="wpool", bufs=1) as wpool,
            tc.tile_pool(name="xpool", bufs=2) as xpool,
            tc.tile_pool(name="chunks", bufs=2) as chpool,
            tc.tile_pool(name="state", bufs=4) as spool,
            tc.tile_pool(name="psx", bufs=2, space="PSUM") as psx_pool,
            tc.tile_pool(name="psz", bufs=2, space="PSUM") as psz_pool,
        ):
            # ---- weight preload (SBUF-resident, fp32) ----
            # layout: W^T as [128, kt*mt*128]; k-row = wt[:, k*mt*128 ...],
            # (k, m) subtile at columns (k*mt + m)*128.
            def load_w(dram, kt, mt, name):
                wt = wpool.tile([128, kt * mt * 128], F16, name=name)
                for k in range(kt):
                    nc.sync.dma_start(
                        wt[:, k * mt * 128:(k + 1) * mt * 128],
                        dram[k * 128:(k + 1) * 128, :],
                    )
                return wt

            def wsl(wt, k, m, mt):
                c = (k * mt + m) * 128
                return wt[:, c:c + 128]

            def wrow(wt, k, mt):
                c = k * mt * 128
                return wt[:, c:c + mt * 128]

            wih0 = load_w(wih0_d, KD, KH, "wih0")
            whh0 = load_w(whh0_d, KH, KH, "whh0")
            wih1 = load_w(wih1_d, KH, KH, "wih1")
            whh1 = load_w(whh1_d, KH, KH, "whh1")
            wfc = load_w(wfc_d, KH, MO, "wfc")
            b0 = wpool.tile([128, KH], F32, name="b0s")
            nc.sync.dma_start(b0[:], b0_d[:])
            b1 = wpool.tile([128, KH], F32, name="b1s")
            nc.sync.dma_start(b1[:], b1_d[:])
            bfc = wpool.tile([128, MO], F32, name="bfcs")
            nc.sync.dma_start(bfc[:], bfc_d[:])
            hz = wpool.tile([128, J, BC], F16, name="hzero")
            nc.vector.memset(hz[:], 0.0)

            def xw_batch(wt, kt, rhs_slices, xwc, bias, half):
                """xwc[:, t, half*4+m, b] = bias[m] + sum_k wt[k,m].T @ rhs

                One two-bank PSUM tile per m; both 512-column splits are
                evacuated by a single ScalarE op."""
                for m in range(KH):
                    ps = psx_pool.tile([128, NS * SPL], F32, name="psxt")
                    for n in range(NS):
                        for k in range(kt):
                            nc.tensor.matmul(
                                ps[:, n * SPL:(n + 1) * SPL],
                                wsl(wt, k, m, KH),
                                rhs_slices(k, n),
                                start=(k == 0),
                                stop=(k == kt - 1),
                            )
                    nc.scalar.activation(
                        xwc[:, :, half * KH + m, :],
                        ps[:].rearrange("p (t b) -> p t b", b=BC),
                        AF.Identity,
                        bias=bias[:, m:m + 1],
                    )

            for _rep in range(reps):
                hc_prev = None
                for c in range(NCH + 1):
                    l0 = c < NCH
                    l1 = c >= 1
                    # ---- per-chunk batched input projections ----
                    xwc = chpool.tile([128, CH, J, BC], F16, name="xwc")
                    if l0:
                        xc = xpool.tile([128, KD * NB], F16, name="xc")
                        for k in range(KD):
                            nc.sync.dma_start(
                                xc[:, k * NB:(k + 1) * NB],
                                xT_d[k * 128:(k + 1) * 128,
                                     c * NB:(c + 1) * NB],
                            )
                        xw_batch(
                            wih0, KD,
                            lambda k, n: xc[:, k * NB + n * SPL:
                                            k * NB + (n + 1) * SPL],
                            xwc, b0, 0,
                        )
                    if l1:
                        prev = hc_prev
                        xw_batch(
                            wih1, KH,
                            lambda k, n: prev[:, n * TPS:(n + 1) * TPS,
                                              k, :],
                            xwc, b1, 1,
                        )

                    hc = chpool.tile([128, CH, J, BC], F16, name="hc")
                    # ---- fused recurrence steps (both layers) ----
                    for tl in range(CH):
                        if tl == 0:
                            h0prev = hz if c == 0 else hc_prev[:, CH - 1]
                            h1prev = hz if c <= 1 else hc_prev[:, CH - 1]
                        else:
                            h0prev = h1prev = hc[:, tl - 1]
                        psz = psz_pool.tile([BC, 2 * 512], F32, name="z")
                        if l0:
                            for k in range(KH):
                                nc.tensor.matmul(
                                    psz[:, 0:512], h0prev[:, k, :],
                                    wrow(whh0, k, KH),
                                    start=(k == 0), stop=(k == KH - 1),
                                )
                        if l1:
                            for k in range(KH):
                                nc.tensor.matmul(
                                    psz[:, 512:1024],
                                    h1prev[:, KH + k, :],
                                    wrow(whh1, k, KH),
                                    start=(k == 0), stop=(k == KH - 1),
                                )
                        zsb = spool.tile([XP, 2 * 512], F16, name="zsb")
                        if l0 and l1:
                            nc.scalar.activation(zsb[0:BC, :], psz[:],
                                                 AF.Identity)
                        elif l0:
                            nc.scalar.activation(zsb[0:BC, 0:512],
                                                 psz[:, 0:512], AF.Identity)
                        else:
                            nc.scalar.activation(zsb[0:BC, 512:1024],
                                                 psz[:, 512:1024],
                                                 AF.Identity)
                        zt = spool.tile([128, J, XP], F16, name="zt")
                        nc.sync.dma_start(zt[:], zsb[:], transpose=True)
                        zpre = spool.tile([128, J, BC], F32, name="zpre")
                        nc.vector.tensor_add(zpre[:], zt[:, :, 0:BC],
                                             xwc[:, tl])
                        nc.scalar.activation(hc[:, tl], zpre[:], AF.Tanh)
                    hc_prev = hc

            # ---- final FC: out^T[o, b] = W_fc[o, :] @ h1_last + b_fc ----
            h1f = hc_prev[:, CH - 1]
            psf = psx_pool.tile([128, MO * BC], F32, name="psxt")
            for m in range(MO):
                for k in range(KH):
                    nc.tensor.matmul(
                        psf[:, m * BC:(m + 1) * BC],
                        wsl(wfc, k, m, MO),
                        h1f[:, KH + k, :],
                        start=(k == 0),
                        stop=(k == KH - 1),
                    )
            outs = spool.tile([128, MO * BC], F32, name="outs")
            for m in range(MO):
                nc.scalar.activation(
                    outs[:, m * BC:(m + 1) * BC],
                    psf[:, m * BC:(m + 1) * BC],
                    AF.Identity,
                    bias=bfc[:, m:m + 1],
                )
            for m in range(MO):
                nc.sync.dma_start(out_d[m * 128:(m + 1) * 128, :],
                                  outs[:, m * BC:(m + 1) * BC])

    nc.compile()
    return nc


def make_in_maps(inputs, T=T):
    """Host-side sharding: full inputs -> per-core input dicts."""
    x = np.asarray(inputs["x"], np.float32)

    def t32(a):
        return np.ascontiguousarray(np.asarray(a, np.float32).T.astype(np.float16))

    shared = {
        "wih0T": t32(inputs["W_ih0"]),
        "whh0T": t32(inputs["W_hh0"]),
        "wih1T": t32(inputs["W_ih1"]),
        "whh1T": t32(inputs["W_hh1"]),
        "wfcT": t32(inputs["W_fc"]),
        "b0": np.ascontiguousarray(
            (np.asarray(inputs["b_ih0"], np.float32)
             + np.asarray(inputs["b_hh0"], np.float32))
            .reshape(KH, 128).T),
        "b1": np.ascontiguousarray(
            (np.asarray(inputs["b_ih1"], np.float32)
             + np.asarray(inputs["b_hh1"], np.float32))
            .reshape(KH, 128).T),
        "bfc": np.ascontiguousarray(
            np.asarray(inputs["b_fc"], np.float32).reshape(MO, 128).T),
    }
    in_maps = []
    for i in range(N_CORES):
        xc = x[i * BC:(i + 1) * BC, :T]  # [BC, T, D]
        xT = np.ascontiguousarray(
            xc.transpose(2, 1, 0).reshape(D, T * BC).astype(np.float16))
        in_maps.append({"xT": xT, **shared})
    return in_maps


def assemble_out(results):
    out = np.empty((B, O), np.float32)
    for i in range(N_CORES):
        out[i * BC:(i + 1) * BC] = results[i]["out"].T
    return out


_NC_CACHE = {}


def kernel(**inputs) -> np.ndarray:
    if "nc" not in _NC_CACHE:
        _NC_CACHE["nc"] = build()
    nc = _NC_CACHE["nc"]
    in_maps = make_in_maps(inputs)
    res = bass_utils.run_bass_kernel_spmd(nc, in_maps, list(range(N_CORES)))
    return assemble_out(res.results)



# revision 5
# speedup vs baseline: 1.8231x; 1.8231x over previous
"""Trainium2 Bass kernel for a 2-layer tanh RNN + final FC.

Reference computation (PyTorch-style RNN, batch_first):
  layer l: h_t = tanh(x_t @ W_ih^T + b_ih + b_hh + h_{t-1} @ W_hh^T)
  out = h1_T @ W_fc^T + b_fc          (final hidden of layer 1)

Shapes: x [64, 1024, 256], H=512, O=256.

Sharding: data-parallel over batch across 8 cores (8 samples/core);
weights replicated.

Per-core design (fp32 matmuls -> single self-loading PE instructions,
fp16 only where the DMA-xbar transpose requires a 2-byte dtype):

* Input projections (x @ W_ih0^T, out0 @ W_ih1^T) are hoisted out of
  the sequential loop and computed as batched matmuls (N=512) over
  128-step chunks, with layer 1 pipelined one chunk behind layer 0.

* The sequential recurrence keeps the hidden state of BOTH layers in
  one fp32 tile hc[:, t] of shape [128, 8, 8]: hc[p, t, j, b] holds
  layer-0 h[b, 128j+p] (chunk c) for j in 0..3 and layer-1 h[b,
  128(j-4)+p] (chunk c-1) for j in 4..7. One fused step computes both
  layers with a minimal instruction count (the backend's cost is
  dominated by fixed per-instruction overheads, not data size):
    - 4+4 fp32 matmuls  z = h @ W_hh^T  (stationary = h j-slice
      [128,8], moving = W_hh^T k-row [128,512]) into ONE two-bank
      PSUM tile [8,1024] (layer 0 cols 0-511, layer 1 cols 512-1023)
    - 1 ScalarE copy into a [16,1024] fp16 staging tile
    - 1 DMA xbar transpose [16,1024] -> [128,8,16]
    - 1 DVE add (+xW chunk) and 1 ScalarE tanh -> hc[:, t]
"""

import sys

if "/opt/trn_rl_repo" not in sys.path:
    sys.path.insert(0, "/opt/trn_rl_repo")

import numpy as np

import concourse.bacc as bacc
import concourse.mybir as mybir
import concourse.tile as tile
from concourse import bass_utils

F16 = mybir.dt.float16
F32 = mybir.dt.float32
F32R = mybir.dt.float32r
AF = mybir.ActivationFunctionType

N_CORES = 8
B, T, D, H, O = 64, 1024, 256, 512, 256
BC = B // N_CORES  # batch per core

CH = 128  # timesteps per chunk
KH = H // 128  # 4
KD = D // 128  # 2
MO = O // 128  # 2
J = 2 * KH  # 8: combined k-tile axis (j 0-3 layer 0, j 4-7 layer 1)
XP = 16  # xbar transpose partition granule


def build(T=T, CH=CH, reps=1):
    """Build the per-core Bass program. reps>1 re-runs the whole body
    (timing amplification only)."""
    NCH = T // CH
    NB = CH * BC  # columns per chunk in (t, b) order
    SPL = min(512, NB)  # matmul free-dim per split (<= one PSUM bank)
    NS = NB // SPL
    TPS = SPL // BC  # timesteps covered per split

    nc = bacc.Bacc("TRN2", target_bir_lowering=False, debug=False,
                   num_devices=N_CORES)

    xT_d = nc.dram_tensor("xT", [D, T * BC], F16, kind="ExternalInput")
    wih0_d = nc.dram_tensor("wih0T", [D, H], F16, kind="ExternalInput")
    whh0_d = nc.dram_tensor("whh0T", [H, H], F16, kind="ExternalInput")
    wih1_d = nc.dram_tensor("wih1T", [H, H], F16, kind="ExternalInput")
    whh1_d = nc.dram_tensor("whh1T", [H, H], F16, kind="ExternalInput")
    wfc_d = nc.dram_tensor("wfcT", [H, O], F16, kind="ExternalInput")
    b0_d = nc.dram_tensor("b0", [128, KH], F32, kind="ExternalInput")
    b1_d = nc.dram_tensor("b1", [128, KH], F32, kind="ExternalInput")
    bfc_d = nc.dram_tensor("bfc", [128, MO], F32, kind="ExternalInput")
    out_d = nc.dram_tensor("out", [O, BC], F32, kind="ExternalOutput")

    with tile.TileContext(nc) as tc:
        with (
            tc.tile_pool(name="wpool", bufs=1) as wpool,
            tc.tile_pool(name="xpool", bufs=2) as xpool,
            tc.tile_pool(name="chunks", bufs=2) as chpool,
            tc.tile_pool(name="state", bufs=4) as spool,
            tc.tile_pool(name="psx", bufs=2, space="PSUM") as psx_pool,
            tc.tile_pool(name="psz", bufs=2, space="PSUM") as psz_pool,
        ):
            # ---- weight preload (SBUF-resident, fp32) ----
            # layout: W^T as [128, kt*mt*128]; k-row = wt[:, k*mt*128 ...],
            # (k, m) subtile at columns (k*mt + m)*128.
            def load_w(dram, kt, mt, name):
                wt = wpool.tile([128, kt * mt * 128], F16, name=name)
                for k in range(kt):
                    nc.sync.dma_start(
                        wt[:, k * mt * 128:(k + 1) * mt * 128],
                        dram[k * 128:(k + 1) * 128, :],
                    )
                return wt

            def wsl(wt, k, m, mt):
                c = (k * mt + m) * 128
                return wt[:, c:c + 128]

            def wrow(wt, k, mt):
                c = k * mt * 128
                return wt[:, c:c + mt * 128]

            wih0 = load_w(wih0_d, KD, KH, "wih0")
            whh0 = load_w(whh0_d, KH, KH, "whh0")
            wih1 = load_w(wih1_d, KH, KH, "wih1")
            whh1 = load_w(whh1_d, KH, KH, "whh1")
            wfc = load_w(wfc_d, KH, MO, "wfc")
            b0 = wpool.tile([128, KH], F32, name="b0s")
            nc.sync.dma_start(b0[:], b0_d[:])
            b1 = wpool.tile([128, KH], F32, name="b1s")
            nc.sync.dma_start(b1[:], b1_d[:])
            bfc = wpool.tile([128, MO], F32, name="bfcs")
            nc.sync.dma_start(bfc[:], bfc_d[:])
            hz = wpool.tile([128, J, BC], F16, name="hzero")
            nc.vector.memset(hz[:], 0.0)

            def xw_batch(wt, kt, rhs_slices, xwc, bias, half):
                """xwc[:, t, half*4+m, b] = bias[m] + sum_k wt[k,m].T @ rhs

                One two-bank PSUM tile per m; both 512-column splits are
                evacuated by a single ScalarE op."""
                for m in range(KH):
                    ps = psx_pool.tile([128, NS * SPL], F32, name="psxt")
                    for n in range(NS):
                        for k in range(kt):
                            nc.tensor.matmul(
                                ps[:, n * SPL:(n + 1) * SPL],
                                wsl(wt, k, m, KH),
                                rhs_slices(k, n),
                                start=(k == 0),
                                stop=(k == kt - 1),
                            )
                    nc.scalar.activation(
                        xwc[:, :, half * KH + m, :],
                        ps[:].rearrange("p (t b) -> p t b", b=BC),
                        AF.Identity,
                        bias=bias[:, m:m + 1],
                    )

            for _rep in range(reps):
                hc_prev = None
                for c in range(NCH + 1):
                    l0 = c < NCH
                    l1 = c >= 1
                    # ---- per-chunk batched input projections ----
                    xwc = chpool.tile([128, CH, J, BC], F16, name="xwc")
                    if l0:
                        xc = xpool.tile([128, KD * NB], F16, name="xc")
                        for k in range(KD):
                            nc.sync.dma_start(
                                xc[:, k * NB:(k + 1) * NB],
                                xT_d[k * 128:(k + 1) * 128,
                                     c * NB:(c + 1) * NB],
                            )
                        xw_batch(
                            wih0, KD,
                            lambda k, n: xc[:, k * NB + n * SPL:
                                            k * NB + (n + 1) * SPL],
                            xwc, b0, 0,
                        )
                    if l1:
                        prev = hc_prev
                        xw_batch(
                            wih1, KH,
                            lambda k, n: prev[:, n * TPS:(n + 1) * TPS,
                                              k, :],
                            xwc, b1, 1,
                        )

                    hc = chpool.tile([128, CH, J, BC], F16, name="hc")
                    # ---- fused recurrence steps (both layers) ----
                    for tl in range(CH):
                        if tl == 0:
                            h0prev = hz if c == 0 else hc_prev[:, CH - 1]
                            h1prev = hz if c <= 1 else hc_prev[:, CH - 1]
                        else:
                            h0prev = h1prev = hc[:, tl - 1]
                        psz = psz_pool.tile([BC, 2 * 512], F32, name="z")
                        if l0:
                            for k in range(KH):
                                nc.tensor.matmul(
                                    psz[:, 0:512], h0prev[:, k, :],
                                    wrow(whh0, k, KH),
                                    start=(k == 0), stop=(k == KH - 1),
                                )
                        if l1:
                            for k in range(KH):
                                nc.tensor.matmul(
                                    psz[:, 512:1024],
                                    h1prev[:, KH + k, :],
                                    wrow(whh1, k, KH),
                                    start=(k == 0), stop=(k == KH - 1),
                                )
                        zsb = spool.tile([XP, 2 * 512], F16, name="zsb")
                        if l0 and l1:
                            nc.scalar.activation(zsb[0:BC, :], psz[:],
                                                 AF.Identity)
                        elif l0:
                            nc.scalar.activation(zsb[0:BC, 0:512],
                                                 psz[:, 0:512], AF.Identity)
                        else:
                            nc.scalar.activation(zsb[0:BC, 512:1024],
                                                 psz[:, 512:1024],
                                                 AF.Identity)
                        zt = spool.tile([128, J, XP], F16, name="zt")
                        nc.sync.dma_start(zt[:], zsb[:], transpose=True)
                        zpre = spool.tile([128, J, BC], F32, name="zpre")
                        nc.vector.tensor_add(zpre[:], zt[:, :, 0:BC],
                                             xwc[:, tl])
                        nc.scalar.activation(hc[:, tl], zpre[:], AF.Tanh)
                    hc_prev = hc

            # ---- final FC: out^T[o, b] = W_fc[o, :] @ h1_last + b_fc ----
            h1f = hc_prev[:, CH - 1]
            psf = psx_pool.tile([128, MO * BC], F32, name="psxt")
            for m in range(MO):
                for k in range(KH):
                    nc.tensor.matmul(
                        psf[:, m * BC:(m + 1) * BC],
                        wsl(wfc, k, m, MO),
                        h1f[:, KH + k, :],
                        start=(k == 0),
                        stop=(k == KH - 1),
                    )
            outs = spool.tile([128, MO * BC], F32, name="outs")
            for m in range(MO):
                nc.scalar.activation(
                    outs[:, m * BC:(m + 1) * BC],
                    psf[:, m * BC:(m + 1) * BC],
                    AF.Identity,
                    bias=bfc[:, m:m + 1],
                )
            for m in range(MO):
                nc.sync.dma_start(out_d[m * 128:(m + 1) * 128, :],
                                  outs[:, m * BC:(m + 1) * BC])

    nc.compile()
    return nc


def make_in_maps(inputs, T=T):
    """Host-side sharding: full inputs -> per-core input dicts."""
    x = np.asarray(inputs["x"], np.float32)

    def t32(a):
        return np.ascontiguousarray(np.asarray(a, np.float32).T.astype(np.float16))

    shared = {
        "wih0T": t32(inputs["W_ih0"]),
        "whh0T": t32(inputs["W_hh0"]),
        "wih1T": t32(inputs["W_ih1"]),
        "whh1T": t32(inputs["W_hh1"]),
        "wfcT": t32(inputs["W_fc"]),
        "b0": np.ascontiguousarray(
            (np.asarray(inputs["b_ih0"], np.float32)
             + np.asarray(inputs["b_hh0"], np.float32))
            .reshape(KH, 128).T),
        "b1": np.ascontiguousarray(
            (np.asarray(inputs["b_ih1"], np.float32)
             + np.asarray(inputs["b_hh1"], np.float32))
            .reshape(KH, 128).T),
        "bfc": np.ascontiguousarray(
            np.asarray(inputs["b_fc"], np.float32).reshape(MO, 128).T),
    }
    in_maps = []
    for i in range(N_CORES):
        xc = x[i * BC:(i + 1) * BC, :T]  # [BC, T, D]
        xT = np.ascontiguousarray(
            xc.transpose(2, 1, 0).reshape(D, T * BC).astype(np.float16))
        in_maps.append({"xT": xT, **shared})
    return in_maps


def assemble_out(results):
    out = np.empty((B, O), np.float32)
    for i in range(N_CORES):
        out[i * BC:(i + 1) * BC] = results[i]["out"].T
    return out


_NC_CACHE = {}


def kernel(**inputs) -> np.ndarray:
    if "nc" not in _NC_CACHE:
        _NC_CACHE["nc"] = build()
    nc = _NC_CACHE["nc"]
    in_maps = make_in_maps(inputs)
    res = bass_utils.run_bass_kernel_spmd(nc, in_maps, list(range(N_CORES)))
    return assemble_out(res.results)

